# revision 9
# baseline (speedup 1.0000x reference)
"""Trainium2 Bass kernel for CustomRationalLayer (v2, bf16 + fused DVE).

Math (B=256 batch, I=512 inputs, O=512 outputs):
    t = tanh(x * tanh_range)                                  (B, I)
    mole[b,o,i] = sum_{p=0..5} mc[o,i,p] t[b,i]^p
    deno[b,o,i] = sum_{q=1..4} dc[o,i,q-1] t[b,i]^q
    out[b,o]    = sum_i mole / (1 + |deno * x[b,i]|)

Strategy: tensor-parallel over O (64 outputs per core).  Per core:
  - power rows [1, t..t^5, t^q*x] computed once in bf16 in an
    (i-partition, b-free) layout, round-tripped through DRAM so a strided
    DMA per phase reloads them in the [20 rows, pairs, B] matmul-rhs
    layout (rows 0-11 mole powers, 12-19 deno u's; row 2r+par = i-half).
  - i processed as 256 pairs j = (i, i+256), 4 pairs per PSUM-tile group.
    Per pair one K=8 bf16 matmul produces deno*x and one K=12 bf16 matmul
    the numerator, into [128, 1024] 2-bank PSUM tiles (pair k at columns
    256k).  ACT abs moves |deno*x| to SBUF f32; a custom fused DVE op
    (seed + 1 Newton step, ~0.2% rel err) computes
       s = mole * recip(1 + |deno*x|)
    in ONE DVE pass (bf16 out), and bf16 identity matmuls accumulate the
    i-sum in PSUM (software-pipelined two groups behind).
Output per core is (64 o, 256 b); host transposes and concatenates.
"""

import numpy as np
import ml_dtypes

import concourse.bass as bass
import concourse.tile as tile
from concourse import bacc, mybir
from concourse import dve_ops as _dve_ops
from concourse.bass_utils import run_bass_kernel_spmd
from concourse.dve_ops import DveOp
from concourse.dve_spec import AluOp, Bin, Spec, Src0, Src1, C0, C1, C2, lower, _has_src1
from concourse.dve_uop import DveOpSpec

B = 256
I = 512
O = 512
NC = 8
OSH = O // NC          # outputs per core
NJ = I // 2            # i-pairs per core
PHJ = 16               # pairs per W-staging phase
NPH = NJ // PHJ
F32 = mybir.dt.float32
BF16 = mybir.dt.bfloat16
AF = mybir.ActivationFunctionType
NPBF16 = ml_dtypes.bfloat16

# 1-Newton-step reciprocal constants (host-tuned: max rel err 2.1e-3 on [1, 300])
C_ADD = 1.0
C_SEED = -0.235
C_NR = 2.0015833333333335

_CACHE = {}


def _fused_recip_mul_op():
    """Custom DVE op: out = Src1 * y1,  y1 ~= 1 / (Src0 + c0).

    Seed y0 = bitwise_not(x) * c1 (exponent-flip trick), one Newton step
    y1 = y0 * (c2 - x*y0).  Registered into dve_ops.OPS so table-gen and
    CoreSim pick it up."""
    name = "RECIP1P_MUL_K45"
    for o in _dve_ops.OPS:
        if o.name == name:
            return o
    _x = Src0 + C0
    _nx = Bin(AluOp.BITWISE_NOT, _x, _x)
    _y0 = _nx * C1
    body = (_y0 * (C2 - _x * _y0)) * Src1

    def _ref(in0, in1, c0, c1, c2):
        x = (in0.astype(np.float32) + np.float32(c0)).astype(np.float32)
        nx = (~x.view(np.int32)).view(np.float32)
        y0 = nx * np.float32(c1)
        y1 = (y0 * (np.float32(c2) - x * y0)).astype(np.float32)
        return (y1 * in1.astype(np.float32)).astype(np.float32)

    spec = Spec(body=body, reference=_ref)
    row = _dve_ops._CUSTOM_DVE_ROW_BASE + len(_dve_ops.OPS)
    uops = lower(spec, ver="v3")
    sha = DveOpSpec(name=name, opcode=row, uops=uops, rd1_en=_has_src1(spec)).sha("v3")
    op = DveOp(name, spec, subdim=False, uops_sha={"v3": sha})
    _dve_ops.OPS.append(op)
    _dve_ops.CUSTOM_DVE_SPECS[name] = spec
    _dve_ops._SUB_OPCODE_FOR_NAME[name] = row
    return op


def _build_bass():
    fused = _fused_recip_mul_op()
    nc = bacc.Bacc("TRN2", target_bir_lowering=False, debug=False, num_devices=NC)

    XT = nc.dram_tensor("xt", [I, B], F32, kind="ExternalInput").ap()
    TRB = nc.dram_tensor("trb", [128, 1], F32, kind="ExternalInput").ap()
    WMD = nc.dram_tensor("wmd", [20, NJ, 128], BF16, kind="ExternalInput").ap()
    ID2 = nc.dram_tensor("id2", [128, OSH], BF16, kind="ExternalInput").ap()
    OUT = nc.dram_tensor("out_y", [OSH, B], F32, kind="ExternalOutput").ap()

    with tile.TileContext(nc) as tc:
        with (
            tc.tile_pool(name="consts", bufs=1) as consts,
            tc.tile_pool(name="powers", bufs=1) as powers,
            tc.tile_pool(name="dramp", bufs=1, space="DRAM") as dramp,
            tc.tile_pool(name="vup", bufs=2) as vup,
            tc.tile_pool(name="wmdp", bufs=2) as wmdp,
            tc.tile_pool(name="zp", bufs=2) as zp,
            tc.tile_pool(name="sp", bufs=6) as sp,
            tc.tile_pool(name="outp", bufs=1) as outp,
            tc.tile_pool(name="pmp", bufs=3, space="PSUM") as pmp,
            tc.tile_pool(name="pdp", bufs=2, space="PSUM") as pdp,
            tc.tile_pool(name="accp", bufs=1, space="PSUM") as accp,
        ):
            id2_s = consts.tile([128, OSH], BF16)
            nc.sync.dma_start(out=id2_s, in_=ID2)
            trb_s = consts.tile([128, 1], F32)
            nc.sync.dma_start(out=trb_s, in_=TRB)

            # x in (i-partition, par, c1, b) layout: i = 256*par + 128*c1 + p
            X = powers.tile([128, 2, 2, B], F32)
            nc.sync.dma_start(
                out=X, in_=XT.rearrange("(par c1 p) b -> p par c1 b", par=2, c1=2)
            )
            Xb = powers.tile([128, 2, 2, B], BF16)
            nc.vector.tensor_copy(Xb, X)

            # PW[:, rt] for rt 0..5: t^rt (row 0 = ones, carries const coef);
            # rt 6..9: u_q = t^q * x.  All bf16.
            PW = powers.tile([128, 10, 2, 2, B], BF16)
            nc.vector.memset(PW[:, 0], 1.0)
            nc.scalar.activation(PW[:, 1], X, AF.Tanh, scale=trb_s[:, 0:1])
            nc.vector.tensor_mul(PW[:, 2], PW[:, 1], PW[:, 1])
            nc.vector.tensor_mul(PW[:, 3], PW[:, 2], PW[:, 1])
            nc.vector.tensor_mul(PW[:, 4], PW[:, 2], PW[:, 2])
            nc.vector.tensor_mul(PW[:, 5], PW[:, 4], PW[:, 1])
            for q in range(4):
                nc.vector.tensor_mul(PW[:, 6 + q], PW[:, 1 + q], Xb)

            # dump in 32-partition strips (contiguous per partition) so each
            # phase's reload only waits on its own strip
            PWDs = []
            for st in range(4):
                pwd = dramp.tile([32, 10, 2, 2, B], BF16, tag=f"pwd{st}")
                nc.sync.dma_start(out=pwd, in_=PW[32 * st : 32 * (st + 1)])
                PWDs.append(pwd.rearrange("p rt par c b -> (rt par) p c b"))

            # acc[o, h, b]: h = pair parity within s4 halves; folded at end
            acc = accp.tile([OSH, 2, B], F32)

            # HAM warm-up: the PE clock sits at 1.2 GHz until a full 4096-cycle
            # activity window (~3.4us) is continuously busy.  A burst of dummy
            # matmuls (dep: only the ones-row memset) runs back-to-back while
            # DVE/ACT compute the power tables, so the real matmul stream
            # starts at 2.4 GHz.  Results land in acc and are discarded by the
            # first real start=True accumulation.
            for _ in range(10):
                nc.tensor.matmul(
                    acc, id2_s, PW[:, 0, 0],
                    start=True, stop=True, skip_group_check=True,
                )

            pending = []
            n_ident = 0

            def flush_ident(limit):
                nonlocal n_ident
                while len(pending) > limit:
                    s2 = pending.pop(0)
                    nc.tensor.matmul(
                        acc, id2_s, s2,
                        start=(n_ident == 0), stop=(n_ident == NJ // 2 - 1),
                        skip_group_check=True,
                    )
                    n_ident += 1

            for ph in range(NPH):
                j0 = PHJ * ph
                c1 = j0 // 128
                po = (j0 % 128) % 32
                st = (j0 % 128) // 32
                wm_s = wmdp.tile([12, PHJ, 128], BF16, tag="wm")
                nc.sync.dma_start(out=wm_s, in_=WMD[0:12, j0 : j0 + PHJ, :])
                wd_s = wmdp.tile([8, PHJ, 128], BF16, tag="wd")
                nc.sync.dma_start(out=wd_s, in_=WMD[12:20, j0 : j0 + PHJ, :])
                with tc.high_priority(offset=400):
                    v2 = vup.tile([12, PHJ, B], BF16, tag="v2")
                    nc.sync.dma_start(
                        out=v2, in_=PWDs[st][0:12, po : po + PHJ, c1, :]
                    )
                    u2 = vup.tile([8, PHJ, B], BF16, tag="u2")
                    nc.sync.dma_start(
                        out=u2, in_=PWDs[st][12:20, po : po + PHJ, c1, :]
                    )

                for g4 in range(PHJ // 4):   # four pairs per abs group
                    pd = pdp.tile([128, 4 * B], F32)
                    pms = []
                    with tc.high_priority(offset=80):
                        for k in range(4):
                            jl = 4 * g4 + k
                            nc.tensor.matmul(
                                pd[:, B * k : B * (k + 1)],
                                wd_s[:, jl, :], u2[:, jl, :],
                                start=True, stop=True,
                            )
                        for h in range(2):   # two pairs per pm / DVE block
                            pm = pmp.tile([128, 2 * B], F32)
                            for k2 in range(2):
                                jl = 4 * g4 + 2 * h + k2
                                nc.tensor.matmul(
                                    pm[:, B * k2 : B * (k2 + 1)],
                                    wm_s[:, jl, :], v2[:, jl, :],
                                    start=True, stop=True,
                                )
                            pms.append(pm)
                    z4 = zp.tile([128, 4 * B], F32, tag="z")
                    nc.scalar.activation(z4, pd, AF.Abs)
                    for h in range(2):
                        s2 = sp.tile([128, 2 * B], BF16, tag="s")
                        nc.vector._custom_dve(
                            fused, out=s2,
                            in0=z4[:, 2 * B * h : 2 * B * (h + 1)], in1=pms[h],
                            s0=C_ADD, s1=C_SEED, imm2=C_NR,
                        )
                        pending.append(s2)
                    flush_ident(4)

            flush_ident(0)

            acc_s = outp.tile([OSH, 2, B], F32)
            nc.scalar.copy(acc_s, acc)
            out_s = outp.tile([OSH, B], F32)
            nc.vector.tensor_add(out_s, acc_s[:, 0], acc_s[:, 1])
            nc.sync.dma_start(out=OUT, in_=out_s)

    nc.compile()
    return nc


def _prep_inputs(x, tanh_range, mole_coef, deno_coef):
    """Host-side prepack -> list of per-core input maps.

    wmd row order for pair j=(i, i+256): row 2r+par = mole coef of power r,
    row 12+2q+par = deno coef of power q+1, for i + 256*par; columns 0:64
    hold par=0 outputs, 64:128 par=1.  Row pair 0/1 (ones row) carries the
    constant mole coef."""
    xt = np.ascontiguousarray(x.T.astype(np.float32))
    trb = np.full((128, 1), np.float32(tanh_range), dtype=np.float32)
    id2 = np.concatenate([np.eye(OSH), np.eye(OSH)], axis=0).astype(NPBF16)
    in_maps = []
    for c in range(NC):
        o0 = OSH * c
        mc = mole_coef[o0 : o0 + OSH]  # (64, 512, 6)
        dc = deno_coef[o0 : o0 + OSH]  # (64, 512, 4)
        wmd = np.zeros((20, NJ, 128), dtype=np.float32)
        for r in range(6):
            wmd[2 * r, :, 0:OSH] = mc[:, 0:NJ, r].T
            wmd[2 * r + 1, :, OSH:128] = mc[:, NJ:I, r].T
        for r in range(4):
            wmd[12 + 2 * r, :, 0:OSH] = dc[:, 0:NJ, r].T
            wmd[12 + 2 * r + 1, :, OSH:128] = dc[:, NJ:I, r].T
        in_maps.append(
            {
                "xt": xt,
                "trb": trb,
                "wmd": wmd.astype(NPBF16),
                "id2": id2,
            }
        )
    return in_maps


def kernel(x, tanh_range, mole_coef, deno_coef):
    x = np.asarray(x, dtype=np.float32)
    mole_coef = np.asarray(mole_coef, dtype=np.float32)
    deno_coef = np.asarray(deno_coef, dtype=np.float32)
    if "nc" not in _CACHE:
        _CACHE["nc"] = _build_bass()
    nc = _CACHE["nc"]
    in_maps = _prep_inputs(x, tanh_range, mole_coef, deno_coef)
    res = run_bass_kernel_spmd(nc, in_maps, list(range(NC)))
    out = np.empty((B, O), dtype=np.float32)
    for c in range(NC):
        out[:, OSH * c : OSH * (c + 1)] = res.results[c]["out_y"].T
    return out
